# revision 3
# baseline (speedup 1.0000x reference)
"""Trainium2 Bass kernel v4 for nn_Critic (LSTM critic over T=512 steps).

v3 + weights baked into the NEFF as constants:
  * All weight-derived SBUF layouts (Ul/Wl tiles in gate order, the fused
    z bias row, stacked Wor, h0-phase weight tiles) are precomputed on the
    host in numpy and embedded via inline_tensor. They become HLO
    constants: materialized on device at executable load, costing nothing
    per call. The per-call input blob carries only the batch tensors
    (action/osc bf16, motion/robot/mu/mean fp32) - ~3.1 MB/core vs 4.6.
  * The per-call axon execute cost is dominated by a ~0.5-1 ms
    per-external-buffer charge plus ~0.75 ms per MB-per-core; one packed
    blob + one output minimizes both.

See kernel3.py / kernel2.py docstrings for the scan/pre structure.
"""

import os
import sys

sys.path.insert(0, "/opt/trn_rl_repo")

from contextlib import ExitStack

import numpy as np

import concourse.bass as bass
import concourse.bacc as bacc
import concourse.mybir as mybir
import concourse.tile as tile
from concourse.masks import make_identity

FP32 = mybir.dt.float32
BF16 = mybir.dt.bfloat16
AF = mybir.ActivationFunctionType
ALU = mybir.AluOpType

B_FULL, T_FULL, A = 256, 512, 32
DM, DR = 64, 128
U = 256
OSC_HALF = 64
NCORES = 8
B = B_FULL // NCORES
XROWS = A + OSC_HALF

SRC_GATE = [0, 1, 3, 2]   # pz block order [i f o g] -> Wl/Ul col chunk [i f g o]

WIN = 16
TQ = 4

NEEDS_WEIGHTS = True      # build_nc(T, inputs) bakes weights into the NEFF


def _prod(shape):
    n = 1
    for s in shape:
        n *= s
    return n


def _blob_layout(T):
    specs = [
        ("action", BF16, (B, T, A)),
        ("osc", BF16, (B, T, OSC_HALF)),
        ("motion_state", FP32, (B, DM)),
        ("robot_state", FP32, (B, DR)),
        ("mu", FP32, (B, A)),
        ("mean", FP32, (B, A)),
    ]
    off = {}
    cur = 0
    for name, dt, shape in specs:
        nb = _prod(shape) * (2 if dt == BF16 else 4)
        off[name] = (cur, dt, shape)
        cur += (nb + 3) // 4 * 4
    return off, cur


def _blob_view(blob, entry):
    o, dt, shape = entry
    nb = _prod(shape) * (2 if dt == BF16 else 4)
    v = blob[o:o + nb].bitcast(dt)
    if len(shape) == 2:
        v = v.rearrange("(a b) -> a b", a=shape[0])
    elif len(shape) == 3:
        v = v.rearrange("(a b c) -> a b c", a=shape[0], b=shape[1])
    return v


def _np_elu(x):
    x = np.asarray(x, np.float64)
    return np.where(x > 0, x, np.expm1(np.minimum(x, 0)))


def _bake_consts(W):
    """Precompute the exact SBUF weight layouts in numpy.

    Returns (bf16 const [128, NBF], fp32 const [128, NF32], col-offset map).
    """
    import ml_dtypes
    bfd = ml_dtypes.bfloat16
    Ul = np.asarray(W["Ul"], np.float32)
    Wl = np.asarray(W["Wl"], np.float32)
    Wor = np.asarray(W["Wor"], np.float32)
    Wo = np.asarray(W["Wo"], np.float32)
    blEff = (np.asarray(W["bl"], np.float64)
             + _np_elu(W["boi"]) @ np.asarray(Wl[XROWS:XROWS + OSC_HALF, :],
                                              np.float64)).astype(np.float32)

    cols = {}
    nbf = 16 * 128 + 8 * 128 + OSC_HALF + 4
    cbf = np.zeros((128, nbf), bfd)
    col = 0
    for k in range(2):
        for c in range(2):
            for g in range(4):
                src = 256 * SRC_GATE[g] + 128 * c
                cbf[:, col:col + 128] = Ul[128 * k:128 * (k + 1),
                                           src:src + 128].astype(bfd)
                cols[f"ul{k}{c}{g}"] = col
                col += 128
    for c in range(2):
        for g in range(4):
            src = 256 * SRC_GATE[g] + 128 * c
            cbf[0:OSC_HALF, col:col + 128] = Wl[A:XROWS, src:src + 128].astype(bfd)
            cbf[OSC_HALF:XROWS, col:col + 128] = Wl[0:A, src:src + 128].astype(bfd)
            cbf[XROWS, col:col + 128] = blEff[src:src + 128].astype(bfd)
            cols[f"wl{c}{g}"] = col
            col += 128
    cbf[0:OSC_HALF, col:col + OSC_HALF] = Wor.astype(bfd)
    cbf[OSC_HALF:128, col:col + OSC_HALF] = Wor.astype(bfd)
    cols["worb2"] = col
    col += OSC_HALF
    cbf[:, col] = Wo[0:128, 0].astype(bfd)
    cbf[:, col + 1] = Wo[128:256, 0].astype(bfd)
    cbf[0, col + 2] = np.float32(np.asarray(W["bo"]).reshape(-1)[0]).astype(bfd)
    cols["wo"] = col
    col += 4
    assert col == nbf

    nf = 2 * 128 + 2 * 128 + 2 * 128 + 8 * 128 + 2 * 128 + 4
    cf = np.zeros((128, nf), np.float32)
    col = 0
    Wm = np.asarray(W["Wm"], np.float32)
    for c in range(2):
        cf[0:DM, col:col + 128] = Wm[:, 128 * c:128 * (c + 1)]
        cf[DM, col:col + 128] = np.asarray(W["bm"], np.float32)[128 * c:128 * (c + 1)]
        cols[f"wm{c}"] = col
        col += 128
    Wr = np.asarray(W["Wr"], np.float32)
    for c in range(2):
        cf[0:DR, col:col + 128] = Wr[:, 128 * c:128 * (c + 1)]
        cols[f"wr{c}"] = col
        col += 128
    for c in range(2):
        cf[0, col:col + 128] = np.asarray(W["br"], np.float32)[128 * c:128 * (c + 1)]
        cols[f"br{c}"] = col
        col += 128
    Wc = np.asarray(W["Wc"], np.float32)
    for k in range(4):
        for c in range(2):
            cf[:, col:col + 128] = Wc[128 * k:128 * (k + 1), 128 * c:128 * (c + 1)]
            cols[f"wc{k}{c}"] = col
            col += 128
    for c in range(2):
        cf[0, col:col + 128] = np.asarray(W["bc"], np.float32)[128 * c:128 * (c + 1)]
        cols[f"bc{c}"] = col
        col += 128
    bor = np.asarray(W["bor"], np.float32)
    cf[0:OSC_HALF, col] = bor
    cf[OSC_HALF:128, col] = bor
    cols["bor"] = col
    col += 4
    assert col == nf
    return cbf, cf, cols


def build_nc(T, inputs):
    nc = bacc.Bacc("TRN2", target_bir_lowering=False, debug=False)

    layout, nbytes = _blob_layout(T)
    blob = nc.dram_tensor("blob", [nbytes], mybir.dt.uint8, kind="ExternalInput").ap()
    d_action = _blob_view(blob, layout["action"])
    d_osc = _blob_view(blob, layout["osc"])
    d_motion = _blob_view(blob, layout["motion_state"])
    d_robot = _blob_view(blob, layout["robot_state"])
    d_mu = _blob_view(blob, layout["mu"])
    d_mean = _blob_view(blob, layout["mean"])
    d_out = nc.dram_tensor("out", [B, 1], FP32, kind="ExternalOutput").ap()

    cbf, cf, cols = _bake_consts(inputs)
    d_cbf = nc.inline_tensor(cbf, name="wconst_bf").ap()
    d_cf = nc.inline_tensor(cf, name="wconst_f32").ap()

    with tile.TileContext(nc) as tc, ExitStack() as ctx:
        _build_body(ctx, tc, T, cols, d_cbf, d_cf,
                    d_action, d_osc, d_motion, d_robot, d_mu, d_mean, d_out)
    nc.finalize()
    return nc


def _build_body(ctx, tc, T, cols, d_cbf, d_cf,
                d_action, d_osc, d_motion, d_robot, d_mu, d_mean, d_out):
    nc = tc.nc
    assert T % WIN == 0 or T < WIN, f"T={T} must be a multiple of {WIN} (or smaller)"

    consts = ctx.enter_context(tc.tile_pool(name="consts", bufs=1))
    weights = ctx.enter_context(tc.tile_pool(name="weights", bufs=1))
    state = ctx.enter_context(tc.tile_pool(name="state", bufs=1))

    ident = consts.tile([128, 128], FP32)
    make_identity(nc, ident)
    ones_bf = consts.tile([1, B], BF16)
    nc.vector.memset(ones_bf, 1.0)

    # ---- baked weights: 2 DMAs, everything else is AP slices ----
    wbf = weights.tile([128, d_cbf.shape[1]], BF16)
    nc.sync.dma_start(out=wbf, in_=d_cbf)
    wf = weights.tile([128, d_cf.shape[1]], FP32)
    nc.sync.dma_start(out=wf, in_=d_cf)

    ulw = [[[wbf[:, cols[f"ul{k}{c}{g}"]:cols[f"ul{k}{c}{g}"] + 128]
             for g in range(4)] for c in range(2)] for k in range(2)]
    wlw = [[wbf[0:XROWS + 1, cols[f"wl{c}{g}"]:cols[f"wl{c}{g}"] + 128]
            for g in range(4)] for c in range(2)]
    worb2 = wbf[:, cols["worb2"]:cols["worb2"] + OSC_HALF]
    wob = [wbf[:, cols["wo"] + c:cols["wo"] + c + 1] for c in range(2)]
    bob = wbf[0:1, cols["wo"] + 2:cols["wo"] + 3]
    wmb = [wf[0:DM + 1, cols[f"wm{c}"]:cols[f"wm{c}"] + 128] for c in range(2)]
    wrb = [wf[0:DR, cols[f"wr{c}"]:cols[f"wr{c}"] + 128] for c in range(2)]
    brb = [wf[0:1, cols[f"br{c}"]:cols[f"br{c}"] + 128] for c in range(2)]
    wcb = [[wf[:, cols[f"wc{k}{c}"]:cols[f"wc{k}{c}"] + 128] for c in range(2)]
           for k in range(4)]
    bcb = [wf[0:1, cols[f"bc{c}"]:cols[f"bc{c}"] + 128] for c in range(2)]
    bor_col = wf[:, cols["bor"]:cols["bor"] + 1]

    # ---------------- persistent state ----------------
    xT = state.tile([XROWS + 1, T * B], BF16)
    nc.vector.memset(xT[XROWS:XROWS + 1, :], 1.0)
    h = state.tile([128, 2 * B], BF16)
    c_st = state.tile([128, 2 * B], FP32)
    hmax = state.tile([128, 2 * B], BF16)
    nc.vector.memset(hmax, -1e30)
    muT4 = state.tile([128, B], FP32)
    meanT4 = state.tile([128, B], FP32)

    with ExitStack() as pre_ctx:
        stage = pre_ctx.enter_context(tc.tile_pool(name="stage", bufs=3))
        ptrans = pre_ctx.enter_context(tc.tile_pool(name="ptrans", bufs=2, space="PSUM"))
        pmm = pre_ctx.enter_context(tc.tile_pool(name="pmm", bufs=2, space="PSUM"))
        posc = pre_ctx.enter_context(tc.tile_pool(name="posc", bufs=2, space="PSUM"))
        scratch = pre_ctx.enter_context(tc.tile_pool(name="scratch", bufs=3))

        # ---- muT4/meanT4: transpose of mu/mean, replicated 4x down ----
        for src, dst in ((d_mu, muT4), (d_mean, meanT4)):
            sb = stage.tile([B, A], FP32, tag="mm_in")
            nc.sync.dma_start(out=sb, in_=src)
            pt = ptrans.tile([A, B], FP32, tag="pt", name="pt_mu")
            nc.tensor.transpose(pt, sb, ident[0:B, 0:B])
            for r in range(4):
                nc.vector.tensor_copy(dst[32 * r:32 * r + 32, :], pt)

        # ---------------- h0 = c0 ----------------
        ones_f = scratch.tile([1, B], FP32)
        nc.vector.memset(ones_f, 1.0)
        motT = scratch.tile([DM + 1, B], FP32)
        mot_sb = scratch.tile([B, DM], FP32)
        nc.sync.dma_start(out=mot_sb, in_=d_motion)
        pt = ptrans.tile([DM, B], FP32, tag="pt", name="pt_mot")
        nc.tensor.transpose(pt, mot_sb, ident[0:B, 0:B])
        nc.vector.tensor_copy(motT[0:DM, :], pt)
        nc.vector.memset(motT[DM:DM + 1, :], 1.0)

        robT = scratch.tile([DR, B], FP32)
        rob_sb = scratch.tile([B, DR], FP32)
        nc.sync.dma_start(out=rob_sb, in_=d_robot)
        pt = ptrans.tile([DR, B], FP32, tag="pt", name="pt_rob")
        nc.tensor.transpose(pt, rob_sb, ident[0:B, 0:B])
        nc.vector.tensor_copy(robT, pt)

        p_ms = pmm.tile([128, 2 * B], FP32, tag="mm", name="p_ms")
        for c in range(2):
            nc.tensor.matmul(p_ms[:, B * c:B * (c + 1)], wmb[c], motT,
                             start=True, stop=True)
        msT = scratch.tile([128, 2 * B], FP32, tag="msT")
        _elu_f32(nc, scratch, msT, p_ms, [128, 2 * B])

        p_rs = pmm.tile([128, 2 * B], FP32, tag="mm", name="p_rs")
        for c in range(2):
            sl = p_rs[:, B * c:B * (c + 1)]
            nc.tensor.matmul(sl, wrb[c], robT, start=True, stop=False)
            nc.tensor.matmul(sl, brb[c], ones_f, start=False, stop=True)
        rsT = scratch.tile([128, 2 * B], FP32, tag="rsT")
        _elu_f32(nc, scratch, rsT, p_rs, [128, 2 * B])

        p_st = pmm.tile([128, 2 * B], FP32, tag="mm", name="p_st")
        for c in range(2):
            sl = p_st[:, B * c:B * (c + 1)]
            nc.tensor.matmul(sl, wcb[0][c], msT[:, 0:B], start=True, stop=False)
            nc.tensor.matmul(sl, wcb[1][c], msT[:, B:2 * B], start=False, stop=False)
            nc.tensor.matmul(sl, wcb[2][c], rsT[:, 0:B], start=False, stop=False)
            nc.tensor.matmul(sl, wcb[3][c], rsT[:, B:2 * B], start=False, stop=False)
            nc.tensor.matmul(sl, bcb[c], ones_f, start=False, stop=True)
        h0_f = scratch.tile([128, 2 * B], FP32, tag="h0f")
        _elu_f32(nc, scratch, h0_f, p_st, [128, 2 * B])
        nc.vector.tensor_copy(c_st, h0_f)
        nc.vector.tensor_copy(h, h0_f)

        # ---------------- preprocessing into xT ----------------
        if not os.environ.get("KERNEL_SKIP_PRE"):
            _pre_phase(nc, tc, T, stage, posc, scratch,
                       d_action, d_osc, worb2, bor_col, muT4, meanT4, xT)
        else:
            nc.vector.memset(xT[0:XROWS, :], 0.01)

    # ---------------- the scan ----------------
    if not os.environ.get("KERNEL_SKIP_SCAN"):
        with ExitStack() as scan_ctx:
            _scan_phase(scan_ctx, tc, T, ulw, wlw, xT, h, c_st, hmax)

    # ---------------- output ----------------
    with ExitStack() as out_ctx:
        pout = out_ctx.enter_context(tc.tile_pool(name="pout", bufs=1, space="PSUM"))
        oscr = out_ctx.enter_context(tc.tile_pool(name="oscr", bufs=1))
        p_out = pout.tile([1, B], FP32)
        nc.tensor.matmul(p_out, wob[0], hmax[:, 0:B], start=True, stop=False)
        nc.tensor.matmul(p_out, wob[1], hmax[:, B:2 * B], start=False, stop=False)
        nc.tensor.matmul(p_out, bob, ones_bf, start=False, stop=True)
        out_sb = oscr.tile([1, B], FP32)
        _elu_f32(nc, oscr, out_sb, p_out, [1, B])
        nc.sync.dma_start(out=d_out.rearrange("b one -> one b"), in_=out_sb)


def _elu_f32(nc, pool, out_ap, y_ap, shape):
    m = pool.tile(shape, FP32, tag="elu_m")
    nc.vector.tensor_scalar_min(m, y_ap, 0.0)
    e = pool.tile(shape, FP32, tag="elu_e")
    nc.scalar.activation(e, m, AF.Exp)
    nc.vector.scalar_tensor_tensor(out_ap, e, -1.0, y_ap, ALU.add, ALU.max)


def _pre_phase(nc, tc, T, stage, posc, scratch,
               d_action, d_osc, worb2, bor_col, muT4, meanT4, xT):
    TH = T // 2
    for bp in range(B // 2):
        b0 = 2 * bp
        oT = [stage.tile([128, TH], BF16, tag="oT", name=f"oT{b0 + db}")
              for db in range(2)]
        for db in range(2):
            nc.sync.dma_start_transpose(
                oT[db], d_osc[b0 + db].rearrange("(a two) f -> a (two f)", two=2))
        pw = posc.tile([128, 2 * TH], FP32, tag="po", name=f"po{bp}")
        for db in range(2):
            for par in range(2):
                sl = slice(OSC_HALF * par, OSC_HALF * (par + 1))
                nc.tensor.matmul(pw[sl, TH * db:TH * (db + 1)],
                                 worb2[sl, :], oT[db][sl, :],
                                 start=True, stop=True)
        z = stage.tile([128, 2 * TH], BF16, tag="oz")
        nc.scalar.activation(z, pw, AF.Identity, bias=bor_col)
        m = stage.tile([128, 2 * TH], BF16, tag="om")
        nc.vector.tensor_scalar_min(m, z, 0.0)
        e = stage.tile([128, 2 * TH], BF16, tag="oe")
        nc.scalar.activation(e, m, AF.Exp)
        for db in range(2):
            for par in range(2):
                sl = slice(OSC_HALF * par, OSC_HALF * (par + 1))
                cs = slice(TH * db, TH * (db + 1))
                dst = xT[0:OSC_HALF, :].rearrange(
                    "p (t x) -> p t x", x=2 * B)[:, :, B * par + b0 + db]
                nc.vector.scalar_tensor_tensor(
                    dst, e[sl, cs], -1.0, z[sl, cs], ALU.add, ALU.max)

    TQ4 = T // 4
    for b in range(B):
        aT = stage.tile([128, TQ4], BF16, tag="aT", name=f"aT{b}")
        nc.sync.dma_start_transpose(
            aT, d_action[b].rearrange("(q four) f -> q (four f)", four=4))
        for r in range(4):
            dst = xT[OSC_HALF:XROWS, :].rearrange(
                "p (q x) -> p q x", x=4 * B)[:, :, B * r + b]
            nc.vector.tensor_scalar(dst, aT[32 * r:32 * r + 32, :],
                                    muT4[32 * r:32 * r + 32, b:b + 1],
                                    meanT4[32 * r:32 * r + 32, b:b + 1],
                                    ALU.mult, ALU.add)


def _scan_phase(ctx, tc, T, ulw, wlw, xT, h, c_st, hmax):
    nc = tc.nc
    pzp = ctx.enter_context(tc.tile_pool(name="pz", bufs=1, space="PSUM"))
    gates = ctx.enter_context(tc.tile_pool(name="gates", bufs=3))

    def emit_window_gemm(w):
        nq = min(WIN, T) // TQ
        tiles = [[pzp.tile([128, TQ * 128], FP32, tag=f"pz{q}{c}",
                           name=f"pz_w{w}_{q}{c}")
                  for c in range(2)] for q in range(nq)]
        for c in range(2):
            for g in range(4):
                for q in range(nq):
                    s0 = WIN * w + TQ * q
                    nc.tensor.matmul(tiles[q][c][:, 128 * g:128 * (g + 1)],
                                     wlw[c][g],
                                     xT[:, B * s0:B * (s0 + TQ)],
                                     start=(g == 0), stop=False,
                                     skip_group_check=True)
        return tiles

    tiles = emit_window_gemm(0)
    for t in range(T):
        w, tl = divmod(t, WIN)
        q, j = divmod(tl, TQ)
        pz0, pz1 = tiles[q][0], tiles[q][1]
        last_j = (j == TQ - 1) or (t == T - 1)
        for k in range(2):
            for c in range(2):
                pz = (pz0, pz1)[c]
                for g in range(4):
                    nc.tensor.matmul(
                        pz[:, 128 * g + 32 * j:128 * g + 32 * (j + 1)],
                        ulw[k][c][g], h[:, B * k:B * (k + 1)],
                        start=False, stop=(k == 1 and g == 3 and last_j),
                        skip_group_check=True)
        if t == T - 1 and T >= WIN:
            pass
        elif tl == WIN - 1:
            next_tiles = emit_window_gemm(w + 1)
        for c in range(2):
            pz = (pz0, pz1)[c]
            S = gates.tile([128, 96], BF16, tag=f"S{c}", name=f"S{c}")
            sig_in = pz.rearrange("p (g x) -> p g x", x=128)[:, 0:3,
                                                            32 * j:32 * (j + 1)]
            nc.scalar.activation(S, sig_in, AF.Sigmoid)
            TG = gates.tile([128, 32], BF16, tag=f"TG{c}", name=f"TG{c}")
            nc.scalar.activation(TG, pz[:, 384 + 32 * j:384 + 32 * (j + 1)], AF.Tanh)
            cs = c_st[:, B * c:B * (c + 1)]
            t2 = gates.tile([128, 32], FP32, tag=f"t2{c}", name=f"t2{c}")
            nc.vector.tensor_mul(t2, S[:, 32:64], cs)
            t1 = gates.tile([128, 32], FP32, tag=f"t1{c}", name=f"t1{c}")
            nc.vector.tensor_mul(t1, S[:, 0:32], TG)
            nc.vector.tensor_add(cs, t1, t2)
            TC = gates.tile([128, 32], BF16, tag=f"TC{c}", name=f"TC{c}")
            nc.scalar.activation(TC, cs, AF.Tanh)
            hs = h[:, B * c:B * (c + 1)]
            nc.vector.tensor_mul(hs, S[:, 64:96], TC)
            nc.vector.tensor_max(hmax[:, B * c:B * (c + 1)],
                                 hmax[:, B * c:B * (c + 1)], hs)
        if tl == WIN - 1 and t != T - 1:
            tiles = next_tiles


# ------------------------------------------------------------------
# host-side entry point
# ------------------------------------------------------------------
_CACHE = {}

WKEYS = ("Wm", "bm", "Wr", "br", "Wc", "bc", "Wor", "bor", "boi",
         "Wl", "bl", "Ul", "Wo", "bo")


def _bf16(a):
    import ml_dtypes
    return np.ascontiguousarray(np.asarray(a, np.float32).astype(ml_dtypes.bfloat16))


def _shard_inputs(inputs, T):
    layout, nbytes = _blob_layout(T)
    full = {
        "action": _bf16(np.asarray(inputs["action"])[:, :T]),
        "osc": _bf16(np.asarray(inputs["osc"])[:, :T, :OSC_HALF]),
    }
    for k in ("motion_state", "robot_state", "mu", "mean"):
        full[k] = np.ascontiguousarray(np.asarray(inputs[k], np.float32))
    in_maps = []
    for i in range(NCORES):
        s = slice(B * i, B * (i + 1))
        blob = np.zeros(nbytes, np.uint8)
        for name, (off, dt, shape) in layout.items():
            raw = np.ascontiguousarray(full[name][s]).view(np.uint8).reshape(-1)
            blob[off:off + raw.size] = raw
        in_maps.append({"blob": blob})
    return in_maps


def _weights_key(inputs):
    import hashlib
    hsh = hashlib.sha1()
    for k in WKEYS:
        hsh.update(np.ascontiguousarray(np.asarray(inputs[k], np.float32)).tobytes())
    return hsh.hexdigest()


def kernel(**inputs) -> np.ndarray:
    from concourse.bass_utils import run_bass_kernel_spmd

    T = int(np.asarray(inputs["action"]).shape[1])
    key = (T, _weights_key(inputs))
    if key not in _CACHE:
        _CACHE[key] = build_nc(T, inputs)
        _CACHE[T] = _CACHE[key]   # alias for harnesses that look up by T
    nc = _CACHE[key]
    in_maps = _shard_inputs(inputs, T)
    res = run_bass_kernel_spmd(nc, in_maps, list(range(NCORES)))
    out = np.concatenate([res.results[i]["out"] for i in range(NCORES)], axis=0)
    return out.astype(np.float32)
